# Initial kernel scaffold
#
"""Trainium2 Bass kernel for nn_Microscope (PSF scatter-add).

Sharding: 8 cores = (b in 0..4) x (h-half in {0,1}).  Each core owns output
rows (b, h_half*128 .. +128) and processes every emitter whose patch rows
intersect its 128-row slab (boundary emitters are duplicated to both
h-halves; each core only writes its own rows, so the output is an exact
partition -- no collectives).

Per core (data-specialized program, compiled at call time):
 - emitters sorted by w, packed 6 per "group" into a [128, 448] bf16 staging
   window (emitter s at partitions 21s..21s+21); 4 group-images per load DMA.
 - ACT: relu + accum_out row-sums.
 - PE+DVE: per 8-group batch, indicator matmuls + reciprocal produce the
   per-emitter scale (i_val * 1e6 / sum) broadcast to [128, 1] slots.
 - DVE: tensor_scalar multiply -> scaled bf16 patches.
 - PE: per-emitter row-routing matmuls.  lhsT = 128-col slice of a static
   block-diagonal shift matrix Z_s (row 21s+k routes to out row k+base; rows
   outside the slab fall outside the slice window = dropped).  rhs = patch
   columns.  out = PSUM, a 7-bank ring over w (bank = 4 w x 128 d),
   accumulating all emitters' contributions per 4-w tile.
 - ACT: evacuate finished psum tiles (crop d) -> SBUF -> DMA to DRAM output.
"""

import threading
from contextlib import ExitStack

import ml_dtypes
import numpy as np

import concourse.bass as bass
import concourse.tile as tile
from concourse import bacc, mybir
from concourse import bass_utils

LAST = None
BF16 = mybir.dt.bfloat16
F32 = mybir.dt.float32
AF = mybir.ActivationFunctionType
BF16NP = ml_dtypes.bfloat16

BS, CH, H, W, D = 4, 1, 256, 256, 64
PH, PW, PD = 21, 21, 21
SCALE_MULT = 10000.0 * 100.0  # folded into i_val
HALF = 128          # h rows per core
G = 6               # emitters per staging group (6*21 = 126 partitions)
GB = 8              # groups per normalization batch
LB = 4              # groups per load DMA
NW = 24             # staging windows
NTILES = 70         # 4-w psum tiles covering w_abs in [-12, 268)
NBANKS = 7          # ring size (8th bank for the normalization chain)
PATCH_COLS = PW * PD  # 441
WIN = 448           # staging window width


def _host_pack(psf_raw, i_val, b, h, w, d):
    cores = []
    for core in range(8):
        b_t, half = core >> 1, core & 1
        lo = half * HALF
        sel = np.where(
            (b == b_t) & (h - PH // 2 <= lo + HALF - 1) & (h + PH // 2 >= lo)
        )[0]
        order = np.argsort(w[sel], kind="stable")
        idx = sel[order]
        ne = len(idx)
        if ne == 0:
            cores.append(None)
            continue
        npad = (-ne) % G
        if npad:
            idx = np.concatenate([idx, np.repeat(idx[:1], npad)])
        ival = i_val[idx].astype(np.float32) * SCALE_MULT
        if npad:
            ival[ne:] = 0.0
        ntot = len(idx)
        ng = ntot // G
        nb = (ng + GB - 1) // GB
        nb4 = (ng + LB - 1) // LB
        # psf packed partition-major per load-batch: [nb4, 126, LB, 441]
        pf = psf_raw[idx].reshape(ng, G * PH, PATCH_COLS).astype(BF16NP)
        psf_packed = np.zeros((nb4, 128, LB, PATCH_COLS), BF16NP)
        for g in range(ng):
            psf_packed[g // LB, 0:G * PH, g % LB, :] = pf[g]
        ival_p = np.zeros((nb, G, GB), np.float32)
        iv = ival.reshape(ng, G)  # [group, slot]
        for g in range(ng):
            ival_p[g // GB, :, g % GB] = iv[g]
        he, we, de = h[idx], w[idx], d[idx]
        base = he.astype(np.int64) - PH // 2 - lo          # in [-20, 127]
        c0 = (127 - base).astype(np.int64)                 # in [0, 147]
        d0 = de.astype(np.int64) + 2                       # in [2, 66)
        t0 = (we.astype(np.int64) + 2) // 4                # first psum tile
        cores.append(dict(ne=ne, ntot=ntot, ng=ng, nb=nb, nb4=nb4,
                          psf=psf_packed, ival=ival_p,
                          c0=c0, d0=d0, w=we.astype(np.int64), t0=t0))
    return cores


def _consts():
    p = np.arange(128)[:, None]
    c = np.arange(288)[None, :]
    zconst = np.stack([
        (((c - (p - 21 * s)) == 127) & (p // 21 == s) & (p < 126)).astype(BF16NP)
        for s in range(G)])                                 # [6, 128, 288]
    ind = ((np.arange(128)[:, None] // 21 == np.arange(G)[None, :])
           & (np.arange(128)[:, None] < 126)).astype(np.float32)   # [128, 6]
    indT = np.ascontiguousarray(ind.T)                      # [6, 128]
    return zconst, ind, indT


def _build_program(cd):
    ng, nb, nb4, ntot, ne = cd["ng"], cd["nb"], cd["nb4"], cd["ntot"], cd["ne"]
    nc = bacc.Bacc("TRN2", target_bir_lowering=False, debug=False)
    psf_d = nc.dram_tensor("psf", [nb4, 128, LB, PATCH_COLS], BF16,
                           kind="ExternalInput").ap()
    ival_d = nc.dram_tensor("ival", [nb, G, GB], F32, kind="ExternalInput").ap()
    z_d = nc.dram_tensor("zconst", [G, 128, 288], BF16, kind="ExternalInput").ap()
    ind_d = nc.dram_tensor("ind", [128, G], F32, kind="ExternalInput").ap()
    indT_d = nc.dram_tensor("indT", [G, 128], F32, kind="ExternalInput").ap()
    out_d = nc.dram_tensor("out", [HALF, W, D], F32, kind="ExternalOutput").ap()

    with tile.TileContext(nc) as tc:
        with ExitStack() as ctx:
            const = ctx.enter_context(tc.tile_pool(name="const", bufs=1))
            evp = ctx.enter_context(tc.tile_pool(name="evp", bufs=4))
            psum = ctx.enter_context(tc.tile_pool(name="psum", bufs=1, space="PSUM"))

            z_t = const.tile([128, G * 288], BF16)
            for s in range(G):
                nc.gpsimd.dma_start(z_t[:, 288 * s:288 * (s + 1)], z_d[s])
            ind_t = const.tile([128, G], F32)
            nc.gpsimd.dma_start(ind_t[:], ind_d[:])
            indT_t = const.tile([G, 128], F32)
            nc.gpsimd.dma_start(indT_t[:], indT_d[:])

            stg = const.tile([128, NW * WIN], BF16)
            rlu = const.tile([128, NW * WIN], BF16)
            scl = const.tile([128, NW * WIN], BF16)

            ring = psum.tile([128, NBANKS * 512], F32)
            ring_r = ring[:].rearrange("p (w d) -> p w d", d=128)
            ps_norm = psum.tile([128, 512], F32)

            rows_t = [const.tile([128, GB], F32, tag=f"rows{i}", name=f"rows{i}")
                      for i in range(2)]
            scale_t = [const.tile([128, GB], F32, tag=f"scale{i}", name=f"scale{i}")
                       for i in range(2)]
            recip_t = [const.tile([G, GB], F32, tag=f"recip{i}", name=f"recip{i}")
                       for i in range(2)]
            ival_t = [const.tile([G, GB], F32, tag=f"ivalt{i}", name=f"ivalt{i}")
                      for i in range(2)]

            def zero_tile(t):
                r = t % NBANKS
                nc.vector.memset(ring[:, 512 * r:512 * (r + 1)], 0.0)

            def evac_tile(t):
                if not (3 <= t <= 66):
                    return
                ev = evp.tile([128, 4 * D], F32, tag="ev", name="ev")
                r = t % NBANKS
                nc.scalar.activation(
                    ev[:].rearrange("p (w d) -> p w d", d=D),
                    ring_r[:, 4 * r:4 * r + 4, 12:76], AF.Copy)
                wb = 4 * (t - 3)
                nc.scalar.dma_start(out_d[:, wb:wb + 4, :], ev[:])

            next_dma = 0       # next load-batch (LB groups) to DMA
            next_relu = 0      # next group to relu
            next_chain_a = 0   # next norm batch, phase A (MM1 + recip)
            next_chain_b = 0   # next norm batch, phase B (MM2 + copy)
            scl_done = set()

            def win(tile_, g):
                return tile_[:, WIN * (g % NW):WIN * (g % NW) + PATCH_COLS]

            def dma_batch():
                nonlocal next_dma
                bi = next_dma
                g0 = bi * LB
                ngrp = min(LB, ng - g0)
                w0 = g0 % NW
                assert w0 + ngrp <= NW
                dst = stg[0:128, WIN * w0:WIN * (w0 + ngrp)].rearrange(
                    "p (g c) -> p g c", c=WIN)[:, :, 0:PATCH_COLS]
                src = psf_d[bi, :, 0:ngrp, :]
                nc.sync.dma_start(dst, src)
                next_dma += 1

            def relu_group(g):
                nonlocal next_relu
                assert g == next_relu
                while next_dma < nb4 and next_dma * LB <= g + 2 * LB:
                    dma_batch()
                rt = rows_t[(g // GB) % 2]
                nc.scalar.activation(win(rlu, g), win(stg, g), AF.Relu,
                                     accum_out=rt[:, (g % GB):(g % GB) + 1])
                next_relu += 1

            def norm_chain_a(k):
                nonlocal next_chain_a
                assert k == next_chain_a
                rt, rct, ivt = rows_t[k % 2], recip_t[k % 2], ival_t[k % 2]
                while next_relu <= min(ng - 1, k * GB + GB - 1):
                    relu_group(next_relu)
                nc.gpsimd.dma_start(ivt[:], ival_d[k])
                nc.tensor.matmul(ps_norm[0:G, (k % 2) * 8:(k % 2) * 8 + GB],
                                 ind_t[:], rt[:],
                                 start=True, stop=True, skip_group_check=True)
                nc.vector.reciprocal(rct[:], ps_norm[0:G, (k % 2) * 8:(k % 2) * 8 + GB])
                nc.vector.tensor_mul(rct[:], rct[:], ivt[:])
                next_chain_a += 1

            def norm_chain_b(k):
                nonlocal next_chain_b
                assert k == next_chain_b
                while next_chain_a <= k:
                    norm_chain_a(next_chain_a)
                sct, rct = scale_t[k % 2], recip_t[k % 2]
                nc.tensor.matmul(
                    ps_norm[0:128, 64 + (k % 2) * 16:64 + (k % 2) * 16 + GB],
                    indT_t[:], rct[:],
                    start=True, stop=True, skip_group_check=True)
                nc.vector.tensor_copy(
                    sct[:], ps_norm[0:128, 64 + (k % 2) * 16:64 + (k % 2) * 16 + GB])
                next_chain_b += 1

            def ensure_scaled(g):
                k = g // GB
                gb = g - k * GB
                # relus a full batch ahead
                batch_end = min(ng - 1, (k + 1) * GB + GB - 1)
                while next_relu <= batch_end:
                    relu_group(next_relu)
                while next_chain_b <= k:
                    norm_chain_b(next_chain_b)
                # chain A for the next batch once 5/8 through this one
                if gb >= 5 and k + 1 < nb:
                    while next_chain_a <= k + 1:
                        norm_chain_a(next_chain_a)
                if g not in scl_done:
                    nc.vector.tensor_scalar(
                        win(scl, g), win(rlu, g),
                        scale_t[k % 2][:, (g % GB):(g % GB) + 1],
                        None, mybir.AluOpType.mult)
                    scl_done.add(g)
                return win(scl, g)

            def emit_emitter_mms(e, sc):
                s = e % G
                c0, d0, we, t0 = (int(cd["c0"][e]), int(cd["d0"][e]),
                                  int(cd["w"][e]), int(cd["t0"][e]))
                lhsT = z_t[:, 288 * s + c0: 288 * s + c0 + 128]
                for t in range(t0, t0 + 6):
                    j0 = max(0, 4 * t - 12 - (we - 10))
                    j1 = min(PW, 4 * t - 8 - (we - 10))
                    nj = j1 - j0
                    if nj <= 0:
                        continue
                    wl = (we - 10 + j0) - (4 * t - 12)
                    r = t % NBANKS
                    rhs = sc[:, j0 * PD:(j0 + nj) * PD].rearrange(
                        "p (j d) -> p j d", d=PD)
                    out = ring_r[:, 4 * r + wl:4 * r + wl + nj, d0:d0 + PD]
                    nc.tensor.matmul(out, lhsT, rhs, start=False, stop=False,
                                     skip_group_check=True)

            # ---- main schedule ----
            t0s = cd["t0"]
            step = 0
            for t in range(min(6, NTILES)):
                zero_tile(t)
            for e in range(ntot):
                if e >= ne:
                    continue
                s = int(t0s[e])
                while step < s:
                    evac_tile(step)
                    step += 1
                    if step + 5 < NTILES:
                        zero_tile(step + 5)
                sc = ensure_scaled(e // G)
                emit_emitter_mms(e, sc)
            while step < NTILES:
                evac_tile(step)
                step += 1
                if step + 5 < NTILES:
                    zero_tile(step + 5)

    nc.compile()
    return nc


def kernel(psf_raw, i_val, b, c, h, w, d):
    psf_raw = np.asarray(psf_raw)
    i_val = np.asarray(i_val)
    b = np.asarray(b); h = np.asarray(h); w = np.asarray(w); d = np.asarray(d)
    n = psf_raw.shape[0]
    psf_flat = psf_raw.reshape(n, PH, PW, PD)

    cores = _host_pack(psf_flat, i_val, b, h, w, d)
    zconst, ind, indT = _consts()

    ncs = [None] * 8
    errs = []

    def build(i):
        try:
            if cores[i] is not None:
                ncs[i] = _build_program(cores[i])
        except BaseException as exc:
            errs.append((i, exc))
            raise

    threads = [threading.Thread(target=build, args=(i,)) for i in range(8)]
    for t in threads:
        t.start()
    for t in threads:
        t.join()
    if errs:
        raise errs[0][1]

    import jax
    devices = jax.devices()
    results = [None] * 8

    def run(i):
        if ncs[i] is None:
            results[i] = {"out": np.zeros((HALF, W, D), np.float32)}
            return
        cd = cores[i]
        in_map = {
            "psf": cd["psf"], "ival": cd["ival"],
            "zconst": zconst, "ind": ind, "indT": indT,
        }
        try:
            with jax.default_device(devices[i]):
                res = bass_utils.run_bass_kernel_spmd(ncs[i], [in_map], core_ids=[0])
            results[i] = res.results[0]
        except BaseException as exc:
            errs.append((i, exc))
            raise

    rthreads = [threading.Thread(target=run, args=(i,)) for i in range(8)]
    for t in rthreads:
        t.start()
    for t in rthreads:
        t.join()
    if errs:
        raise errs[0][1]

    global LAST
    LAST = {"cores": cores, "ncs": ncs, "zconst": zconst, "ind": ind, "indT": indT}

    out = np.zeros((BS, CH, H, W, D), np.float32)
    for core in range(8):
        b_t, half = core >> 1, core & 1
        out[b_t, 0, half * HALF:(half + 1) * HALF] = results[core]["out"]
    return out



# revision 1
# speedup vs baseline: 1.3289x; 1.3289x over previous
"""Trainium2 Bass kernel for nn_Microscope (PSF scatter-add).

Sharding: 8 cores = (b in 0..4) x (h-half in {0,1}).  Each core owns output
rows (b, h_half*128 .. +128) and processes every emitter whose patch rows
intersect its 128-row slab (boundary emitters are duplicated to both
h-halves; each core only writes its own rows, so the output is an exact
partition -- no collectives).

Per core (data-specialized program, compiled at call time):
 - emitters sorted by w, packed 6 per "group" into a [128, 448] bf16 staging
   window (emitter s at partitions 21s..21s+21); 4 group-images per load DMA.
 - ACT: relu + accum_out row-sums.
 - PE+DVE: per 8-group batch, indicator matmuls + reciprocal produce the
   per-emitter scale (i_val * 1e6 / sum) broadcast to [128, 1] slots.
 - DVE: tensor_scalar multiply -> scaled bf16 patches.
 - PE: per-emitter row-routing matmuls.  lhsT = 128-col slice of a static
   block-diagonal shift matrix Z_s (row 21s+k routes to out row k+base; rows
   outside the slab fall outside the slice window = dropped).  rhs = patch
   columns.  out = PSUM, a 7-bank ring over w (bank = 4 w x 128 d),
   accumulating all emitters' contributions per 4-w tile.
 - ACT: evacuate finished psum tiles (crop d) -> SBUF -> DMA to DRAM output.
"""

import threading
from contextlib import ExitStack

import ml_dtypes
import numpy as np

import concourse.bass as bass
import concourse.tile as tile
from concourse import bacc, mybir
from concourse import bass_utils

LAST = None
BF16 = mybir.dt.bfloat16
F32 = mybir.dt.float32
AF = mybir.ActivationFunctionType
BF16NP = ml_dtypes.bfloat16

BS, CH, H, W, D = 4, 1, 256, 256, 64
PH, PW, PD = 21, 21, 21
SCALE_MULT = 10000.0 * 100.0  # folded into i_val
HALF = 128          # h rows per core
G = 6               # emitters per staging group (6*21 = 126 partitions)
GB = 8              # groups per normalization batch
LB = 4              # groups per load DMA
NW = 24             # staging windows
NTILES = 70         # 4-w psum tiles covering w_abs in [-12, 268)
NBANKS = 7          # ring size (8th bank for the normalization chain)
PATCH_COLS = PW * PD  # 441
WIN = 448           # staging window width


def _host_pack(psf_raw, i_val, b, h, w, d):
    cores = []
    for core in range(8):
        b_t, half = core >> 1, core & 1
        lo = half * HALF
        sel = np.where(
            (b == b_t) & (h - PH // 2 <= lo + HALF - 1) & (h + PH // 2 >= lo)
        )[0]
        order = np.argsort(w[sel], kind="stable")
        idx = sel[order]
        ne = len(idx)
        if ne == 0:
            cores.append(None)
            continue
        npad = (-ne) % G
        if npad:
            idx = np.concatenate([idx, np.repeat(idx[:1], npad)])
        ival = i_val[idx].astype(np.float32) * SCALE_MULT
        if npad:
            ival[ne:] = 0.0
        ntot = len(idx)
        ng = ntot // G
        nb = (ng + GB - 1) // GB
        nb4 = (ng + LB - 1) // LB
        # psf packed partition-major per load-batch: [nb4, 126, LB, 441]
        pf = psf_raw[idx].reshape(ng, G * PH, PATCH_COLS).astype(BF16NP)
        psf_packed = np.zeros((nb4, 128, LB, PATCH_COLS), BF16NP)
        for g in range(ng):
            psf_packed[g // LB, 0:G * PH, g % LB, :] = pf[g]
        ival_p = np.zeros((nb, G, GB), np.float32)
        iv = ival.reshape(ng, G)  # [group, slot]
        for g in range(ng):
            ival_p[g // GB, :, g % GB] = iv[g]
        he, we, de = h[idx], w[idx], d[idx]
        base = he.astype(np.int64) - PH // 2 - lo          # in [-20, 127]
        c0 = (127 - base).astype(np.int64)                 # in [0, 147]
        d0 = de.astype(np.int64) + 2                       # in [2, 66)
        t0 = (we.astype(np.int64) + 2) // 4                # first psum tile
        cores.append(dict(ne=ne, ntot=ntot, ng=ng, nb=nb, nb4=nb4,
                          psf=psf_packed, ival=ival_p,
                          c0=c0, d0=d0, w=we.astype(np.int64), t0=t0))
    return cores


def _consts():
    p = np.arange(128)[:, None]
    c = np.arange(288)[None, :]
    zconst = np.stack([
        (((c - (p - 21 * s)) == 127) & (p // 21 == s) & (p < 126)).astype(BF16NP)
        for s in range(G)])                                 # [6, 128, 288]
    ind = ((np.arange(128)[:, None] // 21 == np.arange(G)[None, :])
           & (np.arange(128)[:, None] < 126)).astype(np.float32)   # [128, 6]
    indT = np.ascontiguousarray(ind.T)                      # [6, 128]
    return zconst, ind, indT


def _build_program(cd):
    ng, nb, nb4, ntot, ne = cd["ng"], cd["nb"], cd["nb4"], cd["ntot"], cd["ne"]
    nc = bacc.Bacc("TRN2", target_bir_lowering=False, debug=False)
    psf_d = nc.dram_tensor("psf", [nb4, 128, LB, PATCH_COLS], BF16,
                           kind="ExternalInput").ap()
    ival_d = nc.dram_tensor("ival", [nb, G, GB], F32, kind="ExternalInput").ap()
    z_d = nc.dram_tensor("zconst", [G, 128, 288], BF16, kind="ExternalInput").ap()
    ind_d = nc.dram_tensor("ind", [128, G], F32, kind="ExternalInput").ap()
    indT_d = nc.dram_tensor("indT", [G, 128], F32, kind="ExternalInput").ap()
    out_d = nc.dram_tensor("out", [HALF, W, D], F32, kind="ExternalOutput").ap()

    with tile.TileContext(nc) as tc:
        with ExitStack() as ctx:
            const = ctx.enter_context(tc.tile_pool(name="const", bufs=1))
            evp = ctx.enter_context(tc.tile_pool(name="evp", bufs=4))
            psum = ctx.enter_context(tc.tile_pool(name="psum", bufs=1, space="PSUM"))

            z_t = const.tile([128, G * 288], BF16)
            for s in range(G):
                nc.gpsimd.dma_start(z_t[:, 288 * s:288 * (s + 1)], z_d[s])
            ind_t = const.tile([128, G], F32)
            nc.gpsimd.dma_start(ind_t[:], ind_d[:])
            indT_t = const.tile([G, 128], F32)
            nc.gpsimd.dma_start(indT_t[:], indT_d[:])

            stg = const.tile([128, NW * WIN], BF16)
            rlu = const.tile([128, NW * WIN], BF16)
            scl = const.tile([128, NW * WIN], BF16)

            ring = psum.tile([128, NBANKS * 512], F32)
            ring_r = ring[:].rearrange("p (w d) -> p w d", d=128)
            ps_norm = psum.tile([128, 512], F32)

            rows_t = [const.tile([128, GB], F32, tag=f"rows{i}", name=f"rows{i}")
                      for i in range(2)]
            scale_t = [const.tile([128, GB], F32, tag=f"scale{i}", name=f"scale{i}")
                       for i in range(2)]
            recip_t = [const.tile([G, GB], F32, tag=f"recip{i}", name=f"recip{i}")
                       for i in range(2)]
            ival_t = [const.tile([G, GB], F32, tag=f"ivalt{i}", name=f"ivalt{i}")
                      for i in range(2)]

            def zero_tile(t):
                r = t % NBANKS
                nc.vector.memset(ring[:, 512 * r:512 * (r + 1)], 0.0)

            def evac_tile(t):
                if not (3 <= t <= 66):
                    return
                ev = evp.tile([128, 4 * D], F32, tag="ev", name="ev")
                r = t % NBANKS
                nc.scalar.activation(
                    ev[:].rearrange("p (w d) -> p w d", d=D),
                    ring_r[:, 4 * r:4 * r + 4, 12:76], AF.Copy)
                wb = 4 * (t - 3)
                nc.scalar.dma_start(out_d[:, wb:wb + 4, :], ev[:])

            next_dma = 0       # next load-batch (LB groups) to DMA
            next_relu = 0      # next group to relu
            next_chain_a = 0   # next norm batch, phase A (MM1 + recip)
            next_chain_b = 0   # next norm batch, phase B (MM2 + copy)
            scl_done = set()

            def win(tile_, g):
                return tile_[:, WIN * (g % NW):WIN * (g % NW) + PATCH_COLS]

            def dma_batch():
                nonlocal next_dma
                bi = next_dma
                g0 = bi * LB
                ngrp = min(LB, ng - g0)
                w0 = g0 % NW
                assert w0 + ngrp <= NW
                dst = stg[0:128, WIN * w0:WIN * (w0 + ngrp)].rearrange(
                    "p (g c) -> p g c", c=WIN)[:, :, 0:PATCH_COLS]
                src = psf_d[bi, :, 0:ngrp, :]
                nc.sync.dma_start(dst, src)
                next_dma += 1

            def relu_group(g):
                nonlocal next_relu
                assert g == next_relu
                while next_dma < nb4 and next_dma * LB <= g + 2 * LB:
                    dma_batch()
                rt = rows_t[(g // GB) % 2]
                nc.scalar.activation(win(rlu, g), win(stg, g), AF.Relu,
                                     accum_out=rt[:, (g % GB):(g % GB) + 1])
                next_relu += 1

            def norm_chain_a(k):
                nonlocal next_chain_a
                assert k == next_chain_a
                rt, rct, ivt = rows_t[k % 2], recip_t[k % 2], ival_t[k % 2]
                while next_relu <= min(ng - 1, k * GB + GB - 1):
                    relu_group(next_relu)
                nc.gpsimd.dma_start(ivt[:], ival_d[k])
                nc.tensor.matmul(ps_norm[0:G, (k % 2) * 8:(k % 2) * 8 + GB],
                                 ind_t[:], rt[:],
                                 start=True, stop=True, skip_group_check=True)
                nc.vector.reciprocal(rct[:], ps_norm[0:G, (k % 2) * 8:(k % 2) * 8 + GB])
                nc.vector.tensor_mul(rct[:], rct[:], ivt[:])
                next_chain_a += 1

            def norm_chain_b(k):
                nonlocal next_chain_b
                assert k == next_chain_b
                while next_chain_a <= k:
                    norm_chain_a(next_chain_a)
                sct, rct = scale_t[k % 2], recip_t[k % 2]
                nc.tensor.matmul(
                    ps_norm[0:128, 64 + (k % 2) * 16:64 + (k % 2) * 16 + GB],
                    indT_t[:], rct[:],
                    start=True, stop=True, skip_group_check=True)
                nc.vector.tensor_copy(
                    sct[:], ps_norm[0:128, 64 + (k % 2) * 16:64 + (k % 2) * 16 + GB])
                next_chain_b += 1

            def ensure_scaled(g):
                k = g // GB
                gb = g - k * GB
                # relus a full batch ahead
                batch_end = min(ng - 1, (k + 1) * GB + GB - 1)
                while next_relu <= batch_end:
                    relu_group(next_relu)
                while next_chain_b <= k:
                    norm_chain_b(next_chain_b)
                # chain A for the next batch once 5/8 through this one
                if gb >= 5 and k + 1 < nb:
                    while next_chain_a <= k + 1:
                        norm_chain_a(next_chain_a)
                if g not in scl_done:
                    nc.vector.tensor_scalar(
                        win(scl, g), win(rlu, g),
                        scale_t[k % 2][:, (g % GB):(g % GB) + 1],
                        None, mybir.AluOpType.mult)
                    scl_done.add(g)
                return win(scl, g)

            def emit_emitter_mms(e, sc):
                s = e % G
                c0, d0, we, t0 = (int(cd["c0"][e]), int(cd["d0"][e]),
                                  int(cd["w"][e]), int(cd["t0"][e]))
                lhsT = z_t[:, 288 * s + c0: 288 * s + c0 + 128]
                for t in range(t0, t0 + 6):
                    j0 = max(0, 4 * t - 12 - (we - 10))
                    j1 = min(PW, 4 * t - 8 - (we - 10))
                    nj = j1 - j0
                    if nj <= 0:
                        continue
                    wl = (we - 10 + j0) - (4 * t - 12)
                    r = t % NBANKS
                    rhs = sc[:, j0 * PD:(j0 + nj) * PD].rearrange(
                        "p (j d) -> p j d", d=PD)
                    out = ring_r[:, 4 * r + wl:4 * r + wl + nj, d0:d0 + PD]
                    nc.tensor.matmul(out, lhsT, rhs, start=False, stop=False,
                                     skip_group_check=True)

            # ---- main schedule ----
            t0s = cd["t0"]
            step = 0
            for t in range(min(6, NTILES)):
                zero_tile(t)
            for e in range(ntot):
                if e >= ne:
                    continue
                s = int(t0s[e])
                while step < s:
                    evac_tile(step)
                    step += 1
                    if step + 5 < NTILES:
                        zero_tile(step + 5)
                sc = ensure_scaled(e // G)
                emit_emitter_mms(e, sc)
            while step < NTILES:
                evac_tile(step)
                step += 1
                if step + 5 < NTILES:
                    zero_tile(step + 5)

    nc.compile()
    return nc


def kernel(psf_raw, i_val, b, c, h, w, d):
    psf_raw = np.asarray(psf_raw)
    i_val = np.asarray(i_val)
    b = np.asarray(b); h = np.asarray(h); w = np.asarray(w); d = np.asarray(d)
    n = psf_raw.shape[0]
    psf_flat = psf_raw.reshape(n, PH, PW, PD)

    cores = _host_pack(psf_flat, i_val, b, h, w, d)
    zconst, ind, indT = _consts()

    ncs = [None] * 8
    errs = []

    def build(i):
        try:
            if cores[i] is not None:
                ncs[i] = _build_program(cores[i])
        except BaseException as exc:
            errs.append((i, exc))
            raise

    threads = [threading.Thread(target=build, args=(i,)) for i in range(8)]
    for t in threads:
        t.start()
    for t in threads:
        t.join()
    if errs:
        raise errs[0][1]

    import jax
    devices = jax.devices()
    results = [None] * 8

    def run(i):
        if ncs[i] is None:
            results[i] = {"out": np.zeros((HALF, W, D), np.float32)}
            return
        cd = cores[i]
        in_map = {
            "psf": cd["psf"], "ival": cd["ival"],
            "zconst": zconst, "ind": ind, "indT": indT,
        }
        try:
            with jax.default_device(devices[i]):
                res = bass_utils.run_bass_kernel_spmd(ncs[i], [in_map], core_ids=[0])
            results[i] = res.results[0]
        except BaseException as exc:
            errs.append((i, exc))
            raise

    rthreads = [threading.Thread(target=run, args=(i,)) for i in range(8)]
    for t in rthreads:
        t.start()
    for t in rthreads:
        t.join()
    if errs:
        raise errs[0][1]

    global LAST
    LAST = {"cores": cores, "ncs": ncs, "zconst": zconst, "ind": ind, "indT": indT}

    out = np.zeros((BS, CH, H, W, D), np.float32)
    for core in range(8):
        b_t, half = core >> 1, core & 1
        out[b_t, 0, half * HALF:(half + 1) * HALF] = results[core]["out"]
    return out

